# revision 1
# baseline (speedup 1.0000x reference)
"""Causal self-attention (B=4, T=2048, C=1024, H=16) on 8 TRN2 NeuronCores.

Sharding: tensor-parallel over batch x head-halves. Core c handles batch
b = c//2 and heads 8*(c%2) .. 8*(c%2)+8. Each core computes its 8 heads'
QKV projection, causal attention, and a partial c_proj contribution
(contracting only its 512 c_in rows). The host sums the two partial
outputs per batch and adds the c_proj bias.

Per-core kernel layout (all matmuls bf16 with fp32 PSUM accumulation):
- x^T kept fully resident in SBUF; V (with interleaved ones columns for
  softmax row-sums) is produced first, then Q^T/K^T per head-pair so the
  PE-dense projection of pair hp+1 overlaps the ACT-bound attention of
  pair hp (keeps the PE HAM-warm).
- S^T = K Q^T per (head-pair, q-block, k-tile); two heads run
  concurrently via PE row-tiling (K=64 at partitions 0 and 64).
- exp on ACT with fused 1/8 scale, restricted on diagonal k-tiles to the
  causally-reachable column range; the full-width {0,1} mask multiply
  (DVE for m<2, GpSimd for m>=2) zeroes everything else.
- att@V with [V_h | 1] stationary gives Y^T[d, q] plus row-sum r[q].
  Unnormalized Y^T+r evicted to SBUF; the r row goes through a DMA
  reshape round-trip ([1,512] -> [128,4] for a fast DVE reciprocal ->
  back) and a DRAM partition-broadcast; normalization is one DVE mul.
- c_proj consumes Y^T tiles directly as lhsT (no transposes anywhere).
"""

import os
import numpy as np

# Tile's fine-grained (subtile) dependency tracker misses a required
# semaphore for some partition-sliced read / column-sliced write combos in
# this kernel (nondeterministic corruption on HW, sim-clean). Coarse
# tile-level deps are correct. Must be set before the first tile() call.
os.environ["BY_DEFAULT_DISABLE_SUBTILE_DEPS"] = "1"

B, T, C, H = 4, 2048, 1024, 16
HL = 8          # heads per core
D = 64          # head dim
NCORES = 8
NTB = 4         # t-blocks of 512 over T
NCHUNK = 8      # c_in chunks of 128 over C
NTT = 16        # t-tiles of 128 over T
VGW = 65        # V group width per head (64 cols + ones col)


def _build_nc():
    import concourse.bass as bass
    import concourse.tile as tile
    from concourse import bacc, mybir

    F32 = mybir.dt.float32
    BF16 = mybir.dt.bfloat16
    EXP = mybir.ActivationFunctionType.Exp

    nc = bacc.Bacc("TRN2", target_bir_lowering=False, debug=False,
                   num_devices=NCORES)

    d_xT = nc.dram_tensor("xT", (C, T), BF16, kind="ExternalInput").ap()
    d_wqk = nc.dram_tensor("wqk", (C, 1024), BF16, kind="ExternalInput").ap()
    d_bqk = nc.dram_tensor("bqk", (1024, 1), F32, kind="ExternalInput").ap()
    d_wv = nc.dram_tensor("wv", (C, 512), BF16, kind="ExternalInput").ap()
    d_bv = nc.dram_tensor("bv", (1, 512), BF16, kind="ExternalInput").ap()
    d_w2 = nc.dram_tensor("w2", (512, C), BF16, kind="ExternalInput").ap()
    d_masks = nc.dram_tensor("masks", (4, 128, 1024), BF16,
                             kind="ExternalInput").ap()
    d_ones = nc.dram_tensor("onesr", (128, 128), BF16,
                            kind="ExternalInput").ap()
    d_out = nc.dram_tensor("out", (T, C), F32, kind="ExternalOutput").ap()

    with tile.TileContext(nc) as tc:
        with tc.tile_pool(name="persist", bufs=1) as persist:
            xts = [persist.tile([128, T], BF16, name=f"xt{c}", tag=f"xt{c}")
                   for c in range(NCHUNK)]
            qt = [persist.tile([128, T], BF16, name=f"qt{g}", tag=f"qt{g}")
                  for g in range(4)]
            kt_ = [persist.tile([128, T], BF16, name=f"kt{g}", tag=f"kt{g}")
                   for g in range(4)]
            vt = [persist.tile([128, HL * VGW], BF16, name=f"v{t}",
                               tag=f"v{t}") for t in range(NTT)]
            ynt = [persist.tile([128, T], BF16, name=f"ynt{g}", tag=f"ynt{g}")
                   for g in range(4)]
            mk = [persist.tile([128, 1024], BF16, name=f"mk{m}", tag=f"mk{m}")
                  for m in range(4)]
            bq = [persist.tile([128, 1], F32, name=f"bq{m}", tag=f"bq{m}")
                  for m in range(8)]
            ones1 = persist.tile([1, 128], BF16, name="ones1", tag="ones1")
            bv_sb = persist.tile([1, 512], BF16, name="bv", tag="bv")
            wqk_sb = [persist.tile([128, 1024], BF16, name=f"wqk{c}",
                                   tag=f"wqk{c}") for c in range(NCHUNK)]
            wv_sb = [persist.tile([128, 512], BF16, name=f"wv{c}",
                                  tag=f"wv{c}") for c in range(NCHUNK)]
            w2_sb = [persist.tile([128, C], BF16, name=f"w2{g}", tag=f"w2{g}")
                     for g in range(4)]

            # input DMAs, ordered to unblock the first matmuls fast
            nc.sync.dma_start(ones1[:], d_ones[0:1, :])
            nc.sync.dma_start(bv_sb[:], d_bv[:])
            for c in range(NCHUNK):
                nc.sync.dma_start(xts[c][:], d_xT[c * 128:(c + 1) * 128, :])
                nc.sync.dma_start(wv_sb[c][:], d_wv[c * 128:(c + 1) * 128, :])
                nc.sync.dma_start(wqk_sb[c][:],
                                  d_wqk[c * 128:(c + 1) * 128, :])
            for m in range(8):
                nc.sync.dma_start(bq[m][:], d_bqk[m * 128:(m + 1) * 128, :])
            for m in range(4):
                nc.sync.dma_start(mk[m][:], d_masks[m])
            for g in range(4):
                nc.sync.dma_start(w2_sb[g][:], d_w2[g * 128:(g + 1) * 128, :])
            for t in range(NTT):
                vg0 = vt[t][:].rearrange("p (g c) -> p g c", g=HL)
                nc.sync.dma_start(
                    vg0[:, :, 64:65],
                    d_ones[:, 0:HL].rearrange("p (g o) -> p g o", o=1))

            with tc.tile_pool(name="pt", bufs=6) as pt_pool, \
                 tc.tile_pool(name="yu", bufs=4) as yu_pool, \
                 tc.tile_pool(name="rq", bufs=4) as rq_pool, \
                 tc.tile_pool(name="bc", bufs=4) as bc_pool, \
                 tc.tile_pool(name="rdram", bufs=8, space="DRAM") as rd_pool, \
                 tc.tile_pool(name="outp", bufs=4) as out_pool, \
                 tc.tile_pool(name="psp", bufs=2, space="PSUM") as ps_p, \
                 tc.tile_pool(name="pss", bufs=2, space="PSUM") as ps_s, \
                 tc.tile_pool(name="psy", bufs=2, space="PSUM") as ps_y:

                # ---- V production (PE-dense warmup) ----
                with nc.named_scope("vproj"):
                    for t_idx in range(NTT):
                        tb, tt = divmod(t_idx, 4)
                        tsl = slice(t_idx * 128, (t_idx + 1) * 128)
                        ps = ps_p.tile([128, 512], F32, name="psp", tag="psp")
                        for c in range(NCHUNK):
                            nc.tensor.matmul(ps[:], xts[c][:, tsl],
                                             wv_sb[c][:],
                                             start=(c == 0), stop=False)
                        nc.tensor.matmul(ps[:], ones1[:], bv_sb[:],
                                         start=False, stop=True)
                        vg = vt[t_idx][:].rearrange("p (g c) -> p g c", g=HL)
                        nc.vector.tensor_copy(
                            vg[:, :, 0:64],
                            ps[:].rearrange("p (g c) -> p g c", g=HL))

                def qk_pair(hp):
                    with nc.named_scope(f"qk{hp}"):
                        for mt in (hp, hp + 4):
                            dst = qt[hp] if mt < 4 else kt_[hp]
                            for tb in range(NTB):
                                ts = slice(tb * 512, (tb + 1) * 512)
                                ps = ps_p.tile([128, 512], F32, name="psp",
                                                tag="psp")
                                for c in range(NCHUNK):
                                    nc.tensor.matmul(
                                        ps[:],
                                        wqk_sb[c][:, mt * 128:(mt + 1) * 128],
                                        xts[c][:, ts],
                                        start=(c == 0),
                                        stop=(c == NCHUNK - 1))
                                nc.vector.tensor_scalar_add(dst[:, ts], ps[:],
                                                            bq[mt][:])

                def attention_pair(hp):
                    with nc.named_scope(f"attn{hp}"):
                        for qb in range(NTB):
                            qs = slice(qb * 512, (qb + 1) * 512)
                            nkt = 4 * qb + 4
                            psY = {}
                            for side in (0, 1):
                                psY[side] = ps_y.tile([VGW, 512], F32,
                                                      name="psy", tag="psy")
                            for ktile in range(nkt):
                                ksl = slice(ktile * 128, (ktile + 1) * 128)
                                m = ktile - 4 * qb
                                # both heads share one 2-bank psS tile; the
                                # two row-tiled S matmuls (rows 0-63 / 64-127)
                                # are adjacent in the PE queue and run
                                # concurrently; one wide exp covers both
                                psS = ps_s.tile([128, 1024], F32,
                                                name="pss", tag="pss")
                                for side, po in ((0, 0), (1, 64)):
                                    nc.tensor.matmul(
                                        psS[:, side * 512:(side + 1) * 512],
                                        kt_[hp][po:po + 64, ksl],
                                        qt[hp][po:po + 64, qs])
                                pt = pt_pool.tile([128, 1024], BF16,
                                                  name="pt", tag="pt")
                                nc.scalar.activation(pt[:], psS[:], EXP,
                                                     scale=0.125)
                                if m >= 0:
                                    nc.vector.tensor_mul(pt[:], pt[:],
                                                         mk[m][:])
                                for side, po in ((0, 0), (1, 64)):
                                    h = 2 * hp + side
                                    nc.tensor.matmul(
                                        psY[side][:],
                                        vt[ktile][:, h * VGW:(h + 1) * VGW],
                                        pt[:, side * 512:(side + 1) * 512],
                                        start=(ktile == 0),
                                        stop=(ktile == nkt - 1))
                            # evict unnormalized; normalize via DMA-chain
                            for side, po in ((0, 0), (1, 64)):
                                yu = yu_pool.tile([VGW, 512], F32, name="yu",
                                                  tag="yu")
                                nc.vector.tensor_copy(yu[:], psY[side][:])
                                rd = rd_pool.tile([1, 512], F32, name="rd",
                                                  tag="rd")
                                nc.sync.dma_start(rd[:], yu[64:65, :])
                                rsq = rq_pool.tile([128, 4], F32, name="rsq",
                                                   tag="rsq")
                                nc.sync.dma_start(
                                    rsq[:],
                                    rd[:].rearrange("o (p f) -> (o p) f",
                                                    p=128))
                                rqr = rq_pool.tile([128, 4], F32, name="rqr",
                                                   tag="rqr")
                                nc.vector.reciprocal(rqr[:], rsq[:])
                                rdr = rd_pool.tile([1, 512], F32, name="rdr",
                                                   tag="rdr")
                                nc.sync.dma_start(
                                    rdr[:].rearrange("o (p f) -> (o p) f",
                                                     p=128),
                                    rqr[:])
                                bc = bc_pool.tile([64, 512], F32, name="bc",
                                                  tag="bc")
                                rd_ap = rdr[:]
                                nc.sync.dma_start(
                                    bc[:],
                                    bass.AP(tensor=rd_ap.tensor,
                                            offset=rd_ap.offset,
                                            ap=[[0, 64]] + list(rd_ap.ap[1:])))
                                nc.vector.tensor_mul(
                                    ynt[hp][po:po + 64, qs],
                                    yu[0:64, :], bc[:])

                def cproj_half(gs, accum):
                    with nc.named_scope(f"cproj{gs[0]}"):
                        for tt in range(NTT):
                            tsl = slice(tt * 128, (tt + 1) * 128)
                            for nb in range(2):
                                nsl = slice(nb * 512, (nb + 1) * 512)
                                ps = ps_p.tile([128, 512], F32, name="psp",
                                               tag="psp")
                                for j, g in enumerate(gs):
                                    nc.tensor.matmul(
                                        ps[:], ynt[g][:, tsl],
                                        w2_sb[g][:, nsl],
                                        start=(j == 0),
                                        stop=(j == len(gs) - 1))
                                ob = out_pool.tile([128, 512], F32,
                                                   name="ob", tag="ob")
                                nc.vector.tensor_copy(ob[:], ps[:])
                                if accum:
                                    nc.gpsimd.dma_start(
                                        d_out[tsl, nsl], ob[:],
                                        accum_op=mybir.AluOpType.add)
                                else:
                                    nc.sync.dma_start(d_out[tsl, nsl], ob[:])

                # interleave: QK(hp+1) overlaps attention(hp); first c_proj
                # half (heads 0-3) fills PE gaps during attention of pairs
                # 2-3; second half accumulates into the output at the end.
                qk_pair(0)
                for hp in range(4):
                    if hp + 1 < 4:
                        qk_pair(hp + 1)
                    if hp == 3:
                        cproj_half((0, 1), False)
                    attention_pair(hp)
                cproj_half((2, 3), True)

    nc.compile()
    return nc


def _make_masks():
    i = np.arange(128)[:, None]
    j = np.arange(512)[None, :]
    singles = [(i <= j - 128 * m) for m in range(4)]
    dup = [np.concatenate([s, s], axis=1) for s in singles]
    return np.stack(dup).astype(np.float32)


def _shard_inputs(x, c_attn_w, c_attn_b, c_proj_w):
    import ml_dtypes
    bf16 = ml_dtypes.bfloat16
    masks = _make_masks().astype(bf16)
    ones = np.ones((128, 128), bf16)
    in_maps = []
    for core in range(NCORES):
        b, half = core // 2, core % 2
        h0 = half * HL
        lo, hi = h0 * D, (h0 + HL) * D
        wq = c_attn_w[:, lo:hi]
        wk = c_attn_w[:, C + lo:C + hi]
        wv = c_attn_w[:, 2 * C + lo:2 * C + hi]
        bqk = np.concatenate([c_attn_b[lo:hi],
                              c_attn_b[C + lo:C + hi]])[:, None]
        in_maps.append({
            "xT": np.ascontiguousarray(x[b].T).astype(bf16),
            "wqk": np.ascontiguousarray(
                np.concatenate([wq, wk], axis=1)).astype(bf16),
            "bqk": np.ascontiguousarray(bqk, np.float32),
            "wv": np.ascontiguousarray(wv).astype(bf16),
            "bv": np.ascontiguousarray(
                c_attn_b[2 * C + lo:2 * C + hi][None, :]).astype(bf16),
            "w2": np.ascontiguousarray(c_proj_w[lo:hi, :]).astype(bf16),
            "masks": masks,
            "onesr": ones,
        })
    return in_maps


def _run(x, c_attn_w, c_attn_b, c_proj_w, c_proj_b, trace=False):
    from concourse import bass_utils
    x = np.asarray(x, np.float32)
    c_attn_w = np.asarray(c_attn_w, np.float32)
    c_attn_b = np.asarray(c_attn_b, np.float32)
    c_proj_w = np.asarray(c_proj_w, np.float32)
    c_proj_b = np.asarray(c_proj_b, np.float32)

    nc = _build_nc()
    in_maps = _shard_inputs(x, c_attn_w, c_attn_b, c_proj_w)
    res = bass_utils.run_bass_kernel_spmd(nc, in_maps,
                                          core_ids=list(range(NCORES)),
                                          trace=trace)
    outs = [res.results[c]["out"] for c in range(NCORES)]
    y = np.stack([outs[2 * b] + outs[2 * b + 1] for b in range(B)])
    y += c_proj_b[None, None, :]
    return y.astype(np.float32), res


def kernel(x, c_attn_w, c_attn_b, c_proj_w, c_proj_b):
    y, _ = _run(x, c_attn_w, c_attn_b, c_proj_w, c_proj_b, trace=False)
    return y

